# revision 23
# baseline (speedup 1.0000x reference)
"""Multi-head attention (B=4,S=2048,H=1024,NH=16,D=64) on 8 trn2 cores.

Sharding: core c = (g, b) with g = c // 4 (head-group of 8 heads = 512 dims,
tensor parallel) and b = c % 4 (batch, data parallel). Each core computes a
partial output (its head-group's contribution to the final projection),
transposed: ot = (attn_out_g @ wo_g)^T of shape [H, S]. Host sums the two
group partials per batch and adds bias.

Math notes (host/device split):
  - k-proj bias bk drops out of softmax (adds a per-query constant along the
    key axis), so it is not applied on device.
  - v-proj bias bv commutes through normalized attention (rows of the score
    matrix sum to 1): its contribution is bv @ wo, folded into the output
    bias on the host.

On-device layout: everything is computed transposed (feature dim on
partitions, sequence on the free axis) so the softmax key-axis lands on
partitions. Scores S^T are built per head as K_h^T(stationary) x Q_h^T,
exp() runs on the scalar engine straight out of PSUM, and the ones-column
appended to V in the AV matmul yields the softmax denominators for free.

Schedule: the attention phase is scalar-engine-bound (exp over the full
score matrix), so all projection work is threaded into its PE slack: the
V projection is emitted inside the first head pair's score loop, the
second query-block's Q projection and the first block's output projection
ride in the middle of the scalar-bound stretch. The AV accumulator is
drained to SBUF immediately after the last AV matmul so the PSUM banks
recycle to the next head pair while the reciprocal/broadcast chain for
softmax normalization runs in its shadow.
"""

import sys

if "/opt/trn_rl_repo" not in sys.path:
    sys.path.insert(0, "/opt/trn_rl_repo")

import numpy as np

B, S, H, NH, D = 4, 2048, 1024, 16, 64
G = 2  # head-group split across cores (tensor parallel axis)
GH = H // G  # 512 dims (8 heads) per group
NCORES = 8
SCALE = 1.0 / float(D) ** 0.5  # 1/8

KT = H // 128  # 8 contraction tiles for projections
MT = GH // 128  # 4 m-tiles = head pairs per group
NQC = S // 512  # 4 sequence chunks of 512
SQ = S // 128  # 16 key-sequence tiles
VW = D + 1  # 65: V columns + ones column per head

_CACHE = {}

# build-time tuning knobs
CFG = {
    "xs_bufs": 4,
    "w_bufs": 16,
    "pt_bufs": 16,
    "mm_bufs": 2,
    "o_bufs": 2,
    "od_bufs": 4,
}


def _build():
    import concourse.tile as tile
    from concourse import bacc, mybir

    F32 = mybir.dt.float32
    F32R = mybir.dt.float16  # all-f16 variant: f16 matmuls everywhere
    BF16 = mybir.dt.float16  # f16: same PE speed as bf16, 3 more mantissa bits
    AF = mybir.ActivationFunctionType
    OP = mybir.AluOpType

    nc = bacc.Bacc("TRN2", target_bir_lowering=False, debug=False)

    xq = nc.dram_tensor("xq", [H, S], F32R, kind="ExternalInput")
    xk = nc.dram_tensor("xk", [H, S], F32R, kind="ExternalInput")
    xv = nc.dram_tensor("xv", [H, S], F32R, kind="ExternalInput")
    wqd = nc.dram_tensor("wq", [H, GH], F32R, kind="ExternalInput")
    wkd = nc.dram_tensor("wk", [H, GH], F32R, kind="ExternalInput")
    wvd = nc.dram_tensor("wv", [H, GH], F32R, kind="ExternalInput")
    wod = nc.dram_tensor("wo", [GH, H], F32R, kind="ExternalInput")
    bqd = nc.dram_tensor("bq", [GH], F32, kind="ExternalInput")
    otd = nc.dram_tensor("ot", [H, S], F32R, kind="ExternalOutput")

    with tile.TileContext(nc) as tc:
        with (
            tc.tile_pool(name="res", bufs=1) as res,
            tc.tile_pool(name="rot", bufs=2) as rot,
            tc.tile_pool(name="psmm", bufs=CFG["mm_bufs"], space="PSUM") as psmm,
            tc.tile_pool(name="pso", bufs=CFG["o_bufs"], space="PSUM") as pso,
            tc.tile_pool(name="dsc", bufs=4, space="DRAM") as dsc,
        ):
            # ---- residents ----
            qhT = [
                res.tile([128, S], F32R, tag=f"qhT{m}", name=f"qhT{m}")
                for m in range(MT)
            ]
            khT = [
                res.tile([128, S], F32R, tag=f"khT{m}", name=f"khT{m}")
                for m in range(MT)
            ]
            oT = [
                res.tile([128, S], BF16, tag=f"oT{t}", name=f"oT{t}")
                for t in range(MT)
            ]
            vaug = res.tile([128, SQ * 8 * VW], BF16, tag="vaug", name="vaug")
            wo_bf = [
                res.tile([128, H], BF16, tag=f"wob{t}", name=f"wob{t}")
                for t in range(MT)
            ]
            bq_sb = res.tile([128, MT], F32, tag="bqsb", name="bq_sb")

            # Batched loaders: one 3D-AP DMA per 8-tile set instead of eight
            # dispatches — the sync engine spends ~600ns per dispatch, and
            # serialized dispatches were gating the kernel head.
            def load_w(wd):
                wt = rot.tile([128, KT * GH], F32R, tag="w", bufs=3, name="w")
                for h in range(4):
                    nc.sync.dma_start(
                        out=wt[:, h * 2 * GH : (h + 1) * 2 * GH].rearrange(
                            "p (kt g) -> p kt g", g=GH
                        ),
                        in_=wd.ap()[h * 256 : (h + 1) * 256, :].rearrange(
                            "(kt p) g -> p kt g", p=128
                        ),
                    )
                return [wt[:, kt * GH : (kt + 1) * GH] for kt in range(KT)]

            def load_strips(xd, qc):
                st = rot.tile([128, KT * 512], F32R, tag="xs", bufs=CFG["xs_bufs"], name="xs")
                for h in range(4):
                    nc.sync.dma_start(
                        out=st[:, h * 1024 : (h + 1) * 1024].rearrange(
                            "p (kt s) -> p kt s", s=512
                        ),
                        in_=xd.ap()[
                            h * 256 : (h + 1) * 256, qc * 512 : (qc + 1) * 512
                        ].rearrange("(kt p) s -> p kt s", p=128),
                    )
                return [st[:, kt * 512 : (kt + 1) * 512] for kt in range(KT)]

            # The K-projection inputs go out first so the PE isn't stuck
            # behind the (slow, latency-bound) staging DMAs at kernel start.
            # Alternate the weight/strip sub-DMAs so the first kt tiles of
            # both land as early as possible.
            wkt = rot.tile([128, KT * GH], F32R, tag="w", bufs=3, name="w")
            xkt = rot.tile([128, KT * 512], F32R, tag="xs", bufs=CFG["xs_bufs"], name="xs")
            for h in range(4):
                nc.sync.dma_start(
                    out=wkt[:, h * 2 * GH : (h + 1) * 2 * GH].rearrange(
                        "p (kt g) -> p kt g", g=GH
                    ),
                    in_=wkd.ap()[h * 256 : (h + 1) * 256, :].rearrange(
                        "(kt p) g -> p kt g", p=128
                    ),
                )
                nc.sync.dma_start(
                    out=xkt[:, h * 1024 : (h + 1) * 1024].rearrange(
                        "p (kt s) -> p kt s", s=512
                    ),
                    in_=xk.ap()[h * 256 : (h + 1) * 256, 0:512].rearrange(
                        "(kt p) s -> p kt s", p=128
                    ),
                )
            wk_sb = [wkt[:, kt * GH : (kt + 1) * GH] for kt in range(KT)]
            xk0 = [xkt[:, kt * 512 : (kt + 1) * 512] for kt in range(KT)]

            xk1 = load_strips(xk, 1)

            # ---- constants / weights staging ----
            nc.sync.dma_start(
                out=bq_sb, in_=bqd.ap().rearrange("(m p) -> p m", p=128)
            )
            for t in range(MT):
                nc.sync.dma_start(
                    out=wo_bf[t], in_=wod.ap()[t * 128 : (t + 1) * 128, :]
                )
            # ones columns of vaug only (V slots are written by the V proj)
            nc.vector.memset(
                vaug.rearrange("p (s c) -> p s c", c=VW)[:, :, D : D + 1], 1.0
            )

            wv_sb = load_w(wvd)

            # ---- V projection for one 128-key tile: vaug[kseq, head*65] ----
            def v_proj_sq(xs, qc, sql):
                sq = qc * 4 + sql
                ps = psmm.tile([128, 512], F32, tag="mm", name=f"psv{sq}")
                for kt in range(KT):
                    nc.tensor.matmul(
                        ps,
                        lhsT=xs[kt][:, sql * 128 : (sql + 1) * 128],
                        rhs=wv_sb[kt],
                        start=(kt == 0),
                        stop=(kt == KT - 1),
                    )
                base = sq * 8 * VW
                for h in range(8):
                    nc.vector.tensor_copy(
                        vaug[:, base + h * VW : base + h * VW + D],
                        ps[:, h * D : (h + 1) * D],
                    )

            # ---- K projection: khT[m] = (xk @ wk)^T slice, f32r ----
            for qc in range(NQC):
                xs = (
                    xk0 if qc == 0
                    else xk1 if qc == 1
                    else load_strips(xk, qc)
                )
                for m in range(MT):
                    ps = psmm.tile([128, 512], F32, tag="mm", name=f"psk{m}")
                    for kt in range(KT):
                        nc.tensor.matmul(
                            ps,
                            lhsT=wk_sb[kt][:, m * 128 : (m + 1) * 128],
                            rhs=xs[kt],
                            start=(kt == 0),
                            stop=(kt == KT - 1),
                        )
                    nc.vector.tensor_copy(khT[m][:, qc * 512 : (qc + 1) * 512], ps)

            # ---- Q projection (+ bias) ----
            wq_sb = load_w(wqd)

            def q_proj_m(xs, qc, m):
                ps = psmm.tile([128, 512], F32, tag="mm", name=f"psq{m}")
                for kt in range(KT):
                    nc.tensor.matmul(
                        ps,
                        lhsT=wq_sb[kt][:, m * 128 : (m + 1) * 128],
                        rhs=xs[kt],
                        start=(kt == 0),
                        stop=(kt == KT - 1),
                    )
                nc.vector.tensor_scalar(
                    qhT[m][:, qc * 512 : (qc + 1) * 512],
                    ps,
                    bq_sb[:, m : m + 1],
                    None,
                    OP.add,
                )

            def q_proj(qc):
                xs = load_strips(xq, qc)
                for m in range(MT):
                    q_proj_m(xs, qc, m)

            # Filler queue: small PE work units dribbled one-per-kt into the
            # attention score loops, where the scalar engine is the bottleneck
            # and the PE has slack. Block-emitting these instead would starve
            # the scalar engine (PE is in-order) or stall the PE at phase
            # boundaries.
            filler = []

            def pump(kt):
                if kt % 2 == 1 and filler:
                    filler.pop(0)()

            # ---- attention for head pair t over a 1024-wide query chunk ----
            # extra(kt) lets the caller thread other PE work (V projection
            # chunks) into the score loop; it is emitted before scores(kt) so
            # the pending-AV pops (4 kt behind) always follow their V chunk.
            def attention(t, qcp, extra=None):
                q0 = qcp * 1024
                ps_o = [
                    pso.tile([VW, 1024], F32, tag="o", name=f"pso{hh}")
                    for hh in range(2)
                ]
                pending = []  # (kt, [pt_h0, pt_h1]) awaiting AV
                def emit_av(kt, pts):
                    for hh in range(2):
                        h_abs = 2 * t + hh
                        vbase = kt * 8 * VW + h_abs * VW
                        for qch in range(2):
                            nc.tensor.matmul(
                                ps_o[hh][:, qch * 512 : (qch + 1) * 512],
                                lhsT=vaug[:, vbase : vbase + VW],
                                rhs=pts[hh][:, qch * 512 : (qch + 1) * 512],
                                start=(kt == 0),
                                stop=(kt == SQ - 1),
                            )
                for kt in range(SQ):
                    if extra is not None:
                        extra(kt)
                    pump(kt)
                    ps_ss = []
                    for hh in range(2):
                        hp = 64 * hh
                        ps_s = psmm.tile([128, 1024], F32, tag="mm", name="pss")
                        ps_ss.append(ps_s)
                        for qch in range(2):
                            nc.tensor.matmul(
                                ps_s[:, qch * 512 : (qch + 1) * 512],
                                lhsT=khT[t][hp : hp + 64, kt * 128 : (kt + 1) * 128],
                                rhs=qhT[t][
                                    hp : hp + 64, q0 + qch * 512 : q0 + (qch + 1) * 512
                                ],
                                start=True,
                                stop=True,
                            )
                    pts = []
                    for hh in range(2):
                        pt_t = rot.tile([128, 1024], BF16, tag="pt", bufs=CFG["pt_bufs"], name="pt")
                        nc.scalar.activation(pt_t, ps_ss[hh], AF.Exp, scale=SCALE)
                        pts.append(pt_t)
                    pending.append((kt, pts))
                    if len(pending) > 3:
                        emit_av(*pending.pop(0))
                for p in pending:
                    emit_av(*p)
                # Drain both accumulators to SBUF: the PSUM banks free as soon
                # as the copies retire, so the next head pair's AV starts
                # while the normalization below is still running. The den rows
                # go to DRAM right behind each drain (DMA cannot read PSUM).
                ods, scs = [], []
                for hh in range(2):
                    od = rot.tile([VW, 1024], F32, tag="od", bufs=CFG["od_bufs"], name="od")
                    nc.vector.tensor_copy(od, ps_o[hh])
                    ods.append(od)
                    sc = dsc.tile([1, 1024], F32, tag="sc", name="sc")
                    nc.sync.dma_start(out=sc, in_=od[D : D + 1, :])
                    scs.append(sc)
                # normalize by the ones-column sums; heads stack on partitions.
                # hh1 first: its path to the out-projection has an extra
                # partition-shift DMA hop, so it gets a head start.
                for hh in (1, 0):
                    od = ods[hh]
                    # Reciprocal cost on DVE scales with free-size, so running
                    # it on the [1,1024] den row costs 6.5us and stalls every
                    # consumer queued behind it. Instead bounce the row through
                    # DRAM into a [64,16] spread, recip there (~0.3us), bounce
                    # back, then broadcast (DRAM-source DMA does the 0-stride
                    # partition reads; SBUF source does not support that).
                    dsp = rot.tile([64, 16], F32, tag="dsp", bufs=2, name="dsp")
                    nc.sync.dma_start(
                        out=dsp, in_=scs[hh][0, :].rearrange("(p f) -> p f", f=16)
                    )
                    rcp = rot.tile([64, 16], F32, tag="rcp2", bufs=2, name="rcp")
                    nc.vector.reciprocal(rcp, dsp)
                    sc2 = dsc.tile([1, 1024], F32, tag="sc2", name="sc2")
                    nc.sync.dma_start(
                        out=sc2[0, :].rearrange("(p f) -> p f", f=16), in_=rcp
                    )
                    bc = rot.tile([64, 1024], F32, tag="bc", bufs=2, name="bc")
                    nc.sync.dma_start(
                        out=bc, in_=sc2[0, :].partition_broadcast(64)
                    )
                    if hh == 0:
                        nc.vector.tensor_tensor(
                            oT[t][0:64, q0 : q0 + 1024],
                            od[0:D, :],
                            bc,
                            OP.mult,
                        )
                    else:
                        # normalized h1 lands on partitions 0-63; DMA shifts it
                        # onto partitions 64-127 of the head-pair tile
                        otn = rot.tile([64, 1024], BF16, tag="otn", bufs=2, name="otn")
                        nc.vector.tensor_tensor(
                            otn, od[0:D, :], bc, OP.mult
                        )
                        nc.sync.dma_start(
                            out=oT[t][64:128, q0 : q0 + 1024], in_=otn
                        )

            # ---- output projection, one 128-row m-tile of a 1024 chunk ----
            def out_proj_m(qp, m):
                ps = psmm.tile([128, 1024], F32, tag="mm", name=f"pso{m}")
                for qch in range(2):
                    for t in range(MT):
                        nc.tensor.matmul(
                            ps[:, qch * 512 : (qch + 1) * 512],
                            lhsT=wo_bf[t][:, m * 128 : (m + 1) * 128],
                            rhs=oT[t][
                                :, qp * 1024 + qch * 512 : qp * 1024 + (qch + 1) * 512
                            ],
                            start=(t == 0),
                            stop=(t == MT - 1),
                        )
                osb = rot.tile([128, 1024], BF16, tag="osb", bufs=4, name="osb")
                nc.vector.tensor_copy(osb, ps)
                nc.sync.dma_start(
                    out=otd.ap()[m * 128 : (m + 1) * 128, qp * 1024 : (qp + 1) * 1024],
                    in_=osb,
                )

            # V projection threaded into head pair 0's score loop at sq (128-
            # key) granularity: unit kt produces vaug tile kt; the earliest AV
            # touching it pops 4 iterations later, safely behind it.
            vxs = [None]

            def v_interleave(kt):
                if kt % 4 == 0:
                    vxs[0] = load_strips(xv, kt // 4)
                v_proj_sq(vxs[0], kt // 4, kt % 4)

            q_proj(0)
            q_proj(1)
            attention(0, 0, extra=v_interleave)
            for qc in (2, 3):
                xs_q = load_strips(xq, qc)
                for m in range(MT):
                    filler.append(
                        lambda xs_q=xs_q, qc=qc, m=m: q_proj_m(xs_q, qc, m)
                    )
            attention(1, 0)
            attention(2, 0)
            attention(3, 0)
            attention(0, 1)
            for m in range(H // 128):
                filler.append(lambda m=m: out_proj_m(0, m))
            attention(1, 1)
            attention(2, 1)
            attention(3, 1)
            while filler:
                filler.pop(0)()
            # Tail out-projection in (partial, finish) pairs: the t0-2
            # accumulation runs during the last head pair's normalization
            # chain; only the t3 matmuls wait on it.
            def op_partial(m):
                ps = psmm.tile([128, 1024], F32, tag="mm", name=f"pst{m}")
                for qch in range(2):
                    for t in range(MT - 1):
                        nc.tensor.matmul(
                            ps[:, qch * 512 : (qch + 1) * 512],
                            lhsT=wo_bf[t][:, m * 128 : (m + 1) * 128],
                            rhs=oT[t][:, 1024 + qch * 512 : 1024 + (qch + 1) * 512],
                            start=(t == 0),
                            stop=False,
                        )
                return ps

            def op_finish(m, ps):
                for qch in range(2):
                    nc.tensor.matmul(
                        ps[:, qch * 512 : (qch + 1) * 512],
                        lhsT=wo_bf[MT - 1][:, m * 128 : (m + 1) * 128],
                        rhs=oT[MT - 1][:, 1024 + qch * 512 : 1024 + (qch + 1) * 512],
                        start=False,
                        stop=True,
                    )
                osb = rot.tile([128, 1024], BF16, tag="osb", bufs=4, name="osb")
                nc.vector.tensor_copy(osb, ps)
                nc.sync.dma_start(
                    out=otd.ap()[m * 128 : (m + 1) * 128, 1024:2048], in_=osb
                )

            pair = []
            for m in range(H // 128):
                pair.append((m, op_partial(m)))
                if len(pair) == 2:
                    for m2, ps2 in pair:
                        op_finish(m2, ps2)
                    pair = []

    nc.compile()
    return nc


def _get_nc():
    if "nc" not in _CACHE:
        _CACHE["nc"] = _build()
    return _CACHE["nc"]


def make_in_maps(q, k, v, wq, wk, wv, wo, bq):
    q = np.asarray(q, np.float32)
    k = np.asarray(k, np.float32)
    v = np.asarray(v, np.float32)
    in_maps = []
    for c in range(NCORES):
        g, b = divmod(c, B)
        sl = slice(g * GH, (g + 1) * GH)
        in_maps.append(
            {
                "xq": np.ascontiguousarray(q[b].T).astype(np.float16),
                "xk": np.ascontiguousarray(k[b].T).astype(np.float16),
                "xv": np.ascontiguousarray(v[b].T).astype(np.float16),
                "wq": np.ascontiguousarray(np.asarray(wq, np.float32)[:, sl]).astype(np.float16),
                "wk": np.ascontiguousarray(np.asarray(wk, np.float32)[:, sl]).astype(np.float16),
                "wv": np.ascontiguousarray(np.asarray(wv, np.float32)[:, sl]).astype(np.float16),
                "wo": np.ascontiguousarray(np.asarray(wo, np.float32)[sl, :]).astype(np.float16),
                "bq": np.ascontiguousarray(np.asarray(bq, np.float32)[sl]),
            }
        )
    return in_maps


def assemble(per_core_ot, bv, wo, bo):
    bo_eff = (
        np.asarray(bo, np.float32)
        + np.asarray(bv, np.float32) @ np.asarray(wo, np.float32)
    )
    out = np.empty((B, S, H), np.float32)
    for b in range(B):
        out[b] = (
            per_core_ot[b].T.astype(np.float32)
            + per_core_ot[B + b].T.astype(np.float32)
            + bo_eff
        )
    return out


def kernel(q, k, v, wq, bq, wk, bk, wv, bv, wo, bo, _trace=False):
    from concourse.bass_utils import run_bass_kernel_spmd

    nc = _get_nc()
    in_maps = make_in_maps(q, k, v, wq, wk, wv, wo, bq)
    res = run_bass_kernel_spmd(
        nc, in_maps, core_ids=list(range(NCORES)), trace=_trace
    )
    _CACHE["last_results"] = res
    outs = [res.results[c]["ot"] for c in range(NCORES)]
    return assemble(outs, bv, wo, bo)


# revision 29
# speedup vs baseline: 1.0066x; 1.0066x over previous
"""Multi-head attention (B=4,S=2048,H=1024,NH=16,D=64) on 8 trn2 cores.

Sharding: core c = (g, b) with g = c // 4 (head-group of 8 heads = 512 dims,
tensor parallel) and b = c % 4 (batch, data parallel). Each core computes a
partial output (its head-group's contribution to the final projection),
transposed: ot = (attn_out_g @ wo_g)^T of shape [H, S]. Host sums the two
group partials per batch and adds bias.

Math notes (host/device split):
  - k-proj bias bk drops out of softmax (adds a per-query constant along the
    key axis), so it is not applied on device.
  - v-proj bias bv commutes through normalized attention (rows of the score
    matrix sum to 1): its contribution is bv @ wo, folded into the output
    bias on the host.

On-device layout: everything is computed transposed (feature dim on
partitions, sequence on the free axis) so the softmax key-axis lands on
partitions. Scores S^T are built per head as K_h^T(stationary) x Q_h^T,
exp() runs on the scalar engine straight out of PSUM, and the ones-column
appended to V in the AV matmul yields the softmax denominators for free.

Schedule: the attention phase is scalar-engine-bound (exp over the full
score matrix), so all projection work is threaded into its PE slack: the
V projection is emitted inside the first head pair's score loop, the
second query-block's Q projection and the first block's output projection
ride in the middle of the scalar-bound stretch. The AV accumulator is
drained to SBUF immediately after the last AV matmul so the PSUM banks
recycle to the next head pair while the reciprocal/broadcast chain for
softmax normalization runs in its shadow.
"""

import sys

if "/opt/trn_rl_repo" not in sys.path:
    sys.path.insert(0, "/opt/trn_rl_repo")

import numpy as np

B, S, H, NH, D = 4, 2048, 1024, 16, 64
G = 2  # head-group split across cores (tensor parallel axis)
GH = H // G  # 512 dims (8 heads) per group
NCORES = 8
SCALE = 1.0 / float(D) ** 0.5  # 1/8

KT = H // 128  # 8 contraction tiles for projections
MT = GH // 128  # 4 m-tiles = head pairs per group
NQC = S // 512  # 4 sequence chunks of 512
SQ = S // 128  # 16 key-sequence tiles
VW = D + 1  # 65: V columns + ones column per head

_CACHE = {}

# build-time tuning knobs
CFG = {
    "xs_bufs": 4,
    "w_bufs": 16,
    "pt_bufs": 14,
    "mm_bufs": 2,
    "o_bufs": 2,
    "od_bufs": 4,
}


def _build():
    import concourse.tile as tile
    from concourse import bacc, mybir

    F32 = mybir.dt.float32
    F32R = mybir.dt.float16  # all-f16 variant: f16 matmuls everywhere
    BF16 = mybir.dt.float16  # f16: same PE speed as bf16, 3 more mantissa bits
    AF = mybir.ActivationFunctionType
    OP = mybir.AluOpType

    nc = bacc.Bacc("TRN2", target_bir_lowering=False, debug=False)

    xq = nc.dram_tensor("xq", [H, S], F32R, kind="ExternalInput")
    xk = nc.dram_tensor("xk", [H, S], F32R, kind="ExternalInput")
    xv = nc.dram_tensor("xv", [H, S], F32R, kind="ExternalInput")
    wqd = nc.dram_tensor("wq", [H, GH], F32R, kind="ExternalInput")
    wkd = nc.dram_tensor("wk", [H, GH], F32R, kind="ExternalInput")
    wvd = nc.dram_tensor("wv", [H, GH], F32R, kind="ExternalInput")
    wod = nc.dram_tensor("wo", [GH, H], F32R, kind="ExternalInput")
    bqd = nc.dram_tensor("bq", [GH], F32, kind="ExternalInput")
    otd = nc.dram_tensor("ot", [H, S], F32R, kind="ExternalOutput")

    with tile.TileContext(nc) as tc:
        with (
            tc.tile_pool(name="res", bufs=1) as res,
            tc.tile_pool(name="rot", bufs=2) as rot,
            tc.tile_pool(name="psmm", bufs=CFG["mm_bufs"], space="PSUM") as psmm,
            tc.tile_pool(name="pso", bufs=CFG["o_bufs"], space="PSUM") as pso,
            tc.tile_pool(name="dsc", bufs=4, space="DRAM") as dsc,
        ):
            # ---- residents ----
            qhT = [
                res.tile([128, S], F32R, tag=f"qhT{m}", name=f"qhT{m}")
                for m in range(MT)
            ]
            khT = [
                res.tile([128, S], F32R, tag=f"khT{m}", name=f"khT{m}")
                for m in range(MT)
            ]
            oT = [
                res.tile([128, S], BF16, tag=f"oT{t}", name=f"oT{t}")
                for t in range(MT)
            ]
            vaug = res.tile([128, SQ * 8 * VW], BF16, tag="vaug", name="vaug")
            wo_bf = [
                res.tile([128, H], BF16, tag=f"wob{t}", name=f"wob{t}")
                for t in range(MT)
            ]
            bq_sb = res.tile([128, MT], F32, tag="bqsb", name="bq_sb")

            # Batched loaders: one 3D-AP DMA per 8-tile set instead of eight
            # dispatches — the sync engine spends ~600ns per dispatch, and
            # serialized dispatches were gating the kernel head.
            def load_w(wd):
                wt = rot.tile([128, KT * GH], F32R, tag="w", bufs=3, name="w")
                for h in range(4):
                    nc.sync.dma_start(
                        out=wt[:, h * 2 * GH : (h + 1) * 2 * GH].rearrange(
                            "p (kt g) -> p kt g", g=GH
                        ),
                        in_=wd.ap()[h * 256 : (h + 1) * 256, :].rearrange(
                            "(kt p) g -> p kt g", p=128
                        ),
                    )
                return [wt[:, kt * GH : (kt + 1) * GH] for kt in range(KT)]

            def load_strips(xd, qc):
                st = rot.tile([128, KT * 512], F32R, tag="xs", bufs=CFG["xs_bufs"], name="xs")
                for h in range(4):
                    nc.sync.dma_start(
                        out=st[:, h * 1024 : (h + 1) * 1024].rearrange(
                            "p (kt s) -> p kt s", s=512
                        ),
                        in_=xd.ap()[
                            h * 256 : (h + 1) * 256, qc * 512 : (qc + 1) * 512
                        ].rearrange("(kt p) s -> p kt s", p=128),
                    )
                return [st[:, kt * 512 : (kt + 1) * 512] for kt in range(KT)]

            # The K-projection inputs go out first so the PE isn't stuck
            # behind the (slow, latency-bound) staging DMAs at kernel start.
            # Alternate the weight/strip sub-DMAs so the first kt tiles of
            # both land as early as possible.
            wkt = rot.tile([128, KT * GH], F32R, tag="w", bufs=3, name="w")
            xkt = rot.tile([128, KT * 512], F32R, tag="xs", bufs=CFG["xs_bufs"], name="xs")
            for h in range(4):
                nc.sync.dma_start(
                    out=wkt[:, h * 2 * GH : (h + 1) * 2 * GH].rearrange(
                        "p (kt g) -> p kt g", g=GH
                    ),
                    in_=wkd.ap()[h * 256 : (h + 1) * 256, :].rearrange(
                        "(kt p) g -> p kt g", p=128
                    ),
                )
                nc.sync.dma_start(
                    out=xkt[:, h * 1024 : (h + 1) * 1024].rearrange(
                        "p (kt s) -> p kt s", s=512
                    ),
                    in_=xk.ap()[h * 256 : (h + 1) * 256, 0:512].rearrange(
                        "(kt p) s -> p kt s", p=128
                    ),
                )
            wk_sb = [wkt[:, kt * GH : (kt + 1) * GH] for kt in range(KT)]
            xk0 = [xkt[:, kt * 512 : (kt + 1) * 512] for kt in range(KT)]

            xk1 = load_strips(xk, 1)

            # ---- constants / weights staging ----
            nc.sync.dma_start(
                out=bq_sb, in_=bqd.ap().rearrange("(m p) -> p m", p=128)
            )
            for t in range(MT):
                nc.sync.dma_start(
                    out=wo_bf[t], in_=wod.ap()[t * 128 : (t + 1) * 128, :]
                )
            # ones columns of vaug only (V slots are written by the V proj)
            nc.vector.memset(
                vaug.rearrange("p (s c) -> p s c", c=VW)[:, :, D : D + 1], 1.0
            )

            wv_sb = load_w(wvd)

            # ---- V projection for one 128-key tile: vaug[kseq, head*65] ----
            def v_proj_sq(xs, qc, sql):
                sq = qc * 4 + sql
                ps = psmm.tile([128, 512], F32, tag="mm", name=f"psv{sq}")
                for kt in range(KT):
                    nc.tensor.matmul(
                        ps,
                        lhsT=xs[kt][:, sql * 128 : (sql + 1) * 128],
                        rhs=wv_sb[kt],
                        start=(kt == 0),
                        stop=(kt == KT - 1),
                    )
                base = sq * 8 * VW
                for h in range(8):
                    nc.vector.tensor_copy(
                        vaug[:, base + h * VW : base + h * VW + D],
                        ps[:, h * D : (h + 1) * D],
                    )

            # ---- K projection: khT[m] = (xk @ wk)^T slice, f32r ----
            for qc in range(NQC):
                xs = (
                    xk0 if qc == 0
                    else xk1 if qc == 1
                    else load_strips(xk, qc)
                )
                for m in range(MT):
                    ps = psmm.tile([128, 512], F32, tag="mm", name=f"psk{m}")
                    for kt in range(KT):
                        nc.tensor.matmul(
                            ps,
                            lhsT=wk_sb[kt][:, m * 128 : (m + 1) * 128],
                            rhs=xs[kt],
                            start=(kt == 0),
                            stop=(kt == KT - 1),
                        )
                    nc.vector.tensor_copy(khT[m][:, qc * 512 : (qc + 1) * 512], ps)

            # ---- Q projection (+ bias) ----
            wq_sb = load_w(wqd)

            def q_proj_m(xs, qc, m):
                ps = psmm.tile([128, 512], F32, tag="mm", name=f"psq{m}")
                for kt in range(KT):
                    nc.tensor.matmul(
                        ps,
                        lhsT=wq_sb[kt][:, m * 128 : (m + 1) * 128],
                        rhs=xs[kt],
                        start=(kt == 0),
                        stop=(kt == KT - 1),
                    )
                nc.vector.tensor_scalar(
                    qhT[m][:, qc * 512 : (qc + 1) * 512],
                    ps,
                    bq_sb[:, m : m + 1],
                    None,
                    OP.add,
                )

            def q_proj(qc):
                xs = load_strips(xq, qc)
                for m in range(MT):
                    q_proj_m(xs, qc, m)

            # Filler queue: small PE work units dribbled one-per-kt into the
            # attention score loops, where the scalar engine is the bottleneck
            # and the PE has slack. Block-emitting these instead would starve
            # the scalar engine (PE is in-order) or stall the PE at phase
            # boundaries.
            filler = []

            def pump(kt):
                if kt % 2 == 1 and filler:
                    filler.pop(0)()

            # ---- attention for head pair t over a 1024-wide query chunk ----
            # extra(kt) lets the caller thread other PE work (V projection
            # chunks) into the score loop; it is emitted before scores(kt) so
            # the pending-AV pops (4 kt behind) always follow their V chunk.
            def attention(t, qcp, extra=None, act_norm=False):
                q0 = qcp * 1024
                ps_o = [
                    pso.tile([VW, 1024], F32, tag="o", name=f"pso{hh}")
                    for hh in range(2)
                ]
                pending = []  # (kt, [pt_h0, pt_h1]) awaiting AV
                def emit_av(kt, pts):
                    for hh in range(2):
                        h_abs = 2 * t + hh
                        vbase = kt * 8 * VW + h_abs * VW
                        for qch in range(2):
                            nc.tensor.matmul(
                                ps_o[hh][:, qch * 512 : (qch + 1) * 512],
                                lhsT=vaug[:, vbase : vbase + VW],
                                rhs=pts[hh][:, qch * 512 : (qch + 1) * 512],
                                start=(kt == 0),
                                stop=(kt == SQ - 1),
                            )
                for kt in range(SQ):
                    if extra is not None:
                        extra(kt)
                    pump(kt)
                    ps_ss = []
                    for hh in range(2):
                        hp = 64 * hh
                        ps_s = psmm.tile([128, 1024], F32, tag="mm", name="pss")
                        ps_ss.append(ps_s)
                        for qch in range(2):
                            nc.tensor.matmul(
                                ps_s[:, qch * 512 : (qch + 1) * 512],
                                lhsT=khT[t][hp : hp + 64, kt * 128 : (kt + 1) * 128],
                                rhs=qhT[t][
                                    hp : hp + 64, q0 + qch * 512 : q0 + (qch + 1) * 512
                                ],
                                start=True,
                                stop=True,
                            )
                    pts = []
                    for hh in range(2):
                        pt_t = rot.tile([128, 1024], BF16, tag="pt", bufs=CFG["pt_bufs"], name="pt")
                        nc.scalar.activation(pt_t, ps_ss[hh], AF.Exp, scale=SCALE)
                        pts.append(pt_t)
                    pending.append((kt, pts))
                    if len(pending) > 3:
                        emit_av(*pending.pop(0))
                for p in pending:
                    emit_av(*p)
                # Fast normalization for the final head pair: reciprocal as
                # exp(-ln(x)) on the (tail-idle) scalar engine straight from
                # PSUM — two fewer DMA-latency hops on the one chain whose
                # latency is exposed. Ln and Exp live in the same activation
                # table set, so this costs no table reloads.
                rws = {}
                if act_norm:
                    for hh in (1, 0):
                        rl = rot.tile([VW, 1024], F32, tag="rw", bufs=2, name="rl")
                        nc.scalar.activation(
                            rl[D : D + 1, :], ps_o[hh][D : D + 1, :], AF.Ln, scale=1.0
                        )
                        rw = rot.tile([VW, 1024], F32, tag="rw", bufs=2, name="rw")
                        nc.scalar.activation(
                            rw[D : D + 1, :], rl[D : D + 1, :], AF.Exp, scale=-1.0
                        )
                        rws[hh] = rw
                # Drain both accumulators to SBUF: the PSUM banks free as soon
                # as the copies retire, so the next head pair's AV starts
                # while the normalization below is still running. The den rows
                # go to DRAM right behind each drain (DMA cannot read PSUM).
                ods, scs = [], []
                for hh in range(2):
                    od = rot.tile([VW, 1024], F32, tag="od", bufs=CFG["od_bufs"], name="od")
                    nc.vector.tensor_copy(od, ps_o[hh])
                    ods.append(od)
                    if not act_norm:
                        sc = dsc.tile([1, 1024], F32, tag="sc", name="sc")
                        nc.sync.dma_start(out=sc, in_=od[D : D + 1, :])
                        scs.append(sc)
                # normalize by the ones-column sums; heads stack on partitions.
                # hh1 first: its path to the out-projection has an extra
                # partition-shift DMA hop, so it gets a head start.
                for hh in (1, 0):
                    od = ods[hh]
                    sc2 = dsc.tile([1, 1024], F32, tag="sc2", name="sc2")
                    if act_norm:
                        nc.sync.dma_start(out=sc2, in_=rws[hh][D : D + 1, :])
                    else:
                        # Reciprocal cost on DVE scales with free-size, so
                        # running it on the [1,1024] den row costs 6.5us and
                        # stalls every consumer queued behind it. Instead
                        # bounce the row through DRAM into a [64,16] spread,
                        # recip there (~0.3us), bounce back, then broadcast
                        # (DRAM-source DMA does the 0-stride partition reads;
                        # SBUF source does not support that).
                        dsp = rot.tile([64, 16], F32, tag="dsp", bufs=2, name="dsp")
                        nc.sync.dma_start(
                            out=dsp,
                            in_=scs[hh][0, :].rearrange("(p f) -> p f", f=16),
                        )
                        rcp = rot.tile([64, 16], F32, tag="rcp2", bufs=2, name="rcp")
                        nc.vector.reciprocal(rcp, dsp)
                        nc.sync.dma_start(
                            out=sc2[0, :].rearrange("(p f) -> p f", f=16), in_=rcp
                        )
                    bc = rot.tile([64, 1024], F32, tag="bc", bufs=2, name="bc")
                    nc.sync.dma_start(
                        out=bc, in_=sc2[0, :].partition_broadcast(64)
                    )
                    if hh == 0:
                        nc.vector.tensor_tensor(
                            oT[t][0:64, q0 : q0 + 1024],
                            od[0:D, :],
                            bc,
                            OP.mult,
                        )
                    else:
                        # normalized h1 lands on partitions 0-63; DMA shifts it
                        # onto partitions 64-127 of the head-pair tile
                        otn = rot.tile([64, 1024], BF16, tag="otn", bufs=2, name="otn")
                        nc.vector.tensor_tensor(
                            otn, od[0:D, :], bc, OP.mult
                        )
                        nc.sync.dma_start(
                            out=oT[t][64:128, q0 : q0 + 1024], in_=otn
                        )

            # ---- output projection, one 128-row m-tile of a 1024 chunk ----
            def out_proj_m(qp, m):
                ps = psmm.tile([128, 1024], F32, tag="mm", name=f"pso{m}")
                for qch in range(2):
                    for t in range(MT):
                        nc.tensor.matmul(
                            ps[:, qch * 512 : (qch + 1) * 512],
                            lhsT=wo_bf[t][:, m * 128 : (m + 1) * 128],
                            rhs=oT[t][
                                :, qp * 1024 + qch * 512 : qp * 1024 + (qch + 1) * 512
                            ],
                            start=(t == 0),
                            stop=(t == MT - 1),
                        )
                osb = rot.tile([128, 1024], BF16, tag="osb", bufs=3, name="osb")
                nc.vector.tensor_copy(osb, ps)
                nc.sync.dma_start(
                    out=otd.ap()[m * 128 : (m + 1) * 128, qp * 1024 : (qp + 1) * 1024],
                    in_=osb,
                )

            # V projection threaded into head pair 0's score loop at sq (128-
            # key) granularity: unit kt produces vaug tile kt; the earliest AV
            # touching it pops 4 iterations later, safely behind it.
            vxs = [None]

            def v_interleave(kt):
                if kt % 4 == 0:
                    vxs[0] = load_strips(xv, kt // 4)
                v_proj_sq(vxs[0], kt // 4, kt % 4)

            q_proj(0)
            q_proj(1)
            attention(0, 0, extra=v_interleave)
            for qc in (2, 3):
                xs_q = load_strips(xq, qc)
                for m in range(MT):
                    filler.append(
                        lambda xs_q=xs_q, qc=qc, m=m: q_proj_m(xs_q, qc, m)
                    )
            attention(1, 0)
            attention(2, 0)
            attention(3, 0)
            attention(0, 1)
            for m in range(H // 128):
                filler.append(lambda m=m: out_proj_m(0, m))
            attention(1, 1)
            attention(2, 1)
            attention(3, 1)
            while filler:
                filler.pop(0)()
            # Tail out-projection in (partial, finish) pairs: the t0-2
            # accumulation runs during the last head pair's normalization
            # chain; only the t3 matmuls wait on it.
            def op_partial(m, pool, tag):
                ps = pool.tile([128, 1024], F32, tag=tag, name=f"pst{m}")
                for qch in range(2):
                    for t in range(MT - 1):
                        nc.tensor.matmul(
                            ps[:, qch * 512 : (qch + 1) * 512],
                            lhsT=wo_bf[t][:, m * 128 : (m + 1) * 128],
                            rhs=oT[t][:, 1024 + qch * 512 : 1024 + (qch + 1) * 512],
                            start=(t == 0),
                            stop=False,
                        )
                return ps

            def op_finish(m, ps):
                for qch in range(2):
                    nc.tensor.matmul(
                        ps[:, qch * 512 : (qch + 1) * 512],
                        lhsT=wo_bf[MT - 1][:, m * 128 : (m + 1) * 128],
                        rhs=oT[MT - 1][:, 1024 + qch * 512 : 1024 + (qch + 1) * 512],
                        start=False,
                        stop=True,
                    )
                osb = rot.tile([128, 1024], BF16, tag="osb", bufs=3, name="osb")
                nc.vector.tensor_copy(osb, ps)
                nc.sync.dma_start(
                    out=otd.ap()[m * 128 : (m + 1) * 128, 1024:2048], in_=osb
                )

            # Four concurrent partial groups (both PSUM pools; the pso slots
            # are free once the final drains retire) keep the PE running
            # through the last normalization chain.
            for base in (0, 4):
                pss = [
                    op_partial(base + 0, psmm, "mm"),
                    op_partial(base + 1, psmm, "mm"),
                    op_partial(base + 2, pso, "o"),
                    op_partial(base + 3, pso, "o"),
                ]
                for j, ps2 in enumerate(pss):
                    op_finish(base + j, ps2)

    nc.compile()
    return nc


def _get_nc():
    if "nc" not in _CACHE:
        _CACHE["nc"] = _build()
    return _CACHE["nc"]


def make_in_maps(q, k, v, wq, wk, wv, wo, bq):
    q = np.asarray(q, np.float32)
    k = np.asarray(k, np.float32)
    v = np.asarray(v, np.float32)
    in_maps = []
    for c in range(NCORES):
        g, b = divmod(c, B)
        sl = slice(g * GH, (g + 1) * GH)
        in_maps.append(
            {
                "xq": np.ascontiguousarray(q[b].T).astype(np.float16),
                "xk": np.ascontiguousarray(k[b].T).astype(np.float16),
                "xv": np.ascontiguousarray(v[b].T).astype(np.float16),
                "wq": np.ascontiguousarray(np.asarray(wq, np.float32)[:, sl]).astype(np.float16),
                "wk": np.ascontiguousarray(np.asarray(wk, np.float32)[:, sl]).astype(np.float16),
                "wv": np.ascontiguousarray(np.asarray(wv, np.float32)[:, sl]).astype(np.float16),
                "wo": np.ascontiguousarray(np.asarray(wo, np.float32)[sl, :]).astype(np.float16),
                "bq": np.ascontiguousarray(np.asarray(bq, np.float32)[sl]),
            }
        )
    return in_maps


def assemble(per_core_ot, bv, wo, bo):
    bo_eff = (
        np.asarray(bo, np.float32)
        + np.asarray(bv, np.float32) @ np.asarray(wo, np.float32)
    )
    out = np.empty((B, S, H), np.float32)
    for b in range(B):
        out[b] = (
            per_core_ot[b].T.astype(np.float32)
            + per_core_ot[B + b].T.astype(np.float32)
            + bo_eff
        )
    return out


def kernel(q, k, v, wq, bq, wk, bk, wv, bv, wo, bo, _trace=False):
    from concourse.bass_utils import run_bass_kernel_spmd

    nc = _get_nc()
    in_maps = make_in_maps(q, k, v, wq, wk, wv, wo, bq)
    res = run_bass_kernel_spmd(
        nc, in_maps, core_ids=list(range(NCORES)), trace=_trace
    )
    _CACHE["last_results"] = res
    outs = [res.results[c]["ot"] for c in range(NCORES)]
    return assemble(outs, bv, wo, bo)


# revision 33
# speedup vs baseline: 1.0375x; 1.0307x over previous
"""Multi-head attention (B=4,S=2048,H=1024,NH=16,D=64) on 8 trn2 cores.

Sharding: core c = (g, b) with g = c // 4 (head-group of 8 heads = 512 dims,
tensor parallel) and b = c % 4 (batch, data parallel). Each core computes a
partial output (its head-group's contribution to the final projection),
transposed: ot = (attn_out_g @ wo_g)^T of shape [H, S]. Host sums the two
group partials per batch and adds bias.

Math notes (host/device split):
  - k-proj bias bk drops out of softmax (adds a per-query constant along the
    key axis), so it is not applied on device.
  - v-proj bias bv commutes through normalized attention (rows of the score
    matrix sum to 1): its contribution is bv @ wo, folded into the output
    bias on the host.

On-device layout: everything is computed transposed (feature dim on
partitions, sequence on the free axis) so the softmax key-axis lands on
partitions. Scores S^T are built per head as K_h^T(stationary) x Q_h^T,
exp() runs on the scalar engine straight out of PSUM, and the ones-column
appended to V in the AV matmul yields the softmax denominators for free.

Schedule: the attention phase is scalar-engine-bound (exp over the full
score matrix), so all projection work is threaded into its PE slack: the
V projection is emitted inside the first head pair's score loop, the
second query-block's Q projection and the first block's output projection
ride in the middle of the scalar-bound stretch. The AV accumulator is
drained to SBUF immediately after the last AV matmul so the PSUM banks
recycle to the next head pair while the reciprocal/broadcast chain for
softmax normalization runs in its shadow.
"""

import sys

if "/opt/trn_rl_repo" not in sys.path:
    sys.path.insert(0, "/opt/trn_rl_repo")

import numpy as np

B, S, H, NH, D = 4, 2048, 1024, 16, 64
G = 2  # head-group split across cores (tensor parallel axis)
GH = H // G  # 512 dims (8 heads) per group
NCORES = 8
SCALE = 1.0 / float(D) ** 0.5  # 1/8

KT = H // 128  # 8 contraction tiles for projections
MT = GH // 128  # 4 m-tiles = head pairs per group
NQC = S // 512  # 4 sequence chunks of 512
SQ = S // 128  # 16 key-sequence tiles
VW = D + 1  # 65: V columns + ones column per head

_CACHE = {}

# build-time tuning knobs
CFG = {
    "xs_bufs": 4,
    "w_bufs": 16,
    "pt_bufs": 14,
    "mm_bufs": 2,
    "o_bufs": 2,
    "od_bufs": 4,
}


def _build():
    import concourse.tile as tile
    from concourse import bacc, mybir

    F32 = mybir.dt.float32
    F32R = mybir.dt.float16  # all-f16 variant: f16 matmuls everywhere
    BF16 = mybir.dt.float16  # f16: same PE speed as bf16, 3 more mantissa bits
    AF = mybir.ActivationFunctionType
    OP = mybir.AluOpType

    nc = bacc.Bacc("TRN2", target_bir_lowering=False, debug=False)

    xq = nc.dram_tensor("xq", [H, S], F32R, kind="ExternalInput")
    xk = nc.dram_tensor("xk", [H, S], F32R, kind="ExternalInput")
    xv = nc.dram_tensor("xv", [H, S], F32R, kind="ExternalInput")
    wqd = nc.dram_tensor("wq", [H, GH], F32R, kind="ExternalInput")
    wkd = nc.dram_tensor("wk", [H, GH], F32R, kind="ExternalInput")
    wvd = nc.dram_tensor("wv", [H, GH], F32R, kind="ExternalInput")
    wod = nc.dram_tensor("wo", [GH, H], F32R, kind="ExternalInput")
    bqd = nc.dram_tensor("bq", [GH], F32, kind="ExternalInput")
    otd = nc.dram_tensor("ot", [H, S], F32R, kind="ExternalOutput")

    with tile.TileContext(nc) as tc:
        with (
            tc.tile_pool(name="res", bufs=1) as res,
            tc.tile_pool(name="rot", bufs=2) as rot,
            tc.tile_pool(name="psmm", bufs=CFG["mm_bufs"], space="PSUM") as psmm,
            tc.tile_pool(name="pso", bufs=CFG["o_bufs"], space="PSUM") as pso,
            tc.tile_pool(name="dsc", bufs=4, space="DRAM") as dsc,
        ):
            # ---- residents ----
            qhT = [
                res.tile([128, S], F32R, tag=f"qhT{m}", name=f"qhT{m}")
                for m in range(MT)
            ]
            khT = [
                res.tile([128, S], F32R, tag=f"khT{m}", name=f"khT{m}")
                for m in range(MT)
            ]
            oT = [
                res.tile([128, S], BF16, tag=f"oT{t}", name=f"oT{t}")
                for t in range(MT)
            ]
            vaug = res.tile([128, SQ * 8 * VW], BF16, tag="vaug", name="vaug")
            wo_bf = [
                res.tile([128, H], BF16, tag=f"wob{t}", name=f"wob{t}")
                for t in range(MT)
            ]
            bq_sb = res.tile([128, MT], F32, tag="bqsb", name="bq_sb")

            # Batched loaders: one 3D-AP DMA per 8-tile set instead of eight
            # dispatches — the sync engine spends ~600ns per dispatch, and
            # serialized dispatches were gating the kernel head.
            def load_w(wd):
                wt = rot.tile([128, KT * GH], F32R, tag="w", bufs=3, name="w")
                for h in range(4):
                    nc.sync.dma_start(
                        out=wt[:, h * 2 * GH : (h + 1) * 2 * GH].rearrange(
                            "p (kt g) -> p kt g", g=GH
                        ),
                        in_=wd.ap()[h * 256 : (h + 1) * 256, :].rearrange(
                            "(kt p) g -> p kt g", p=128
                        ),
                    )
                return [wt[:, kt * GH : (kt + 1) * GH] for kt in range(KT)]

            def load_strips(xd, qc):
                st = rot.tile([128, KT * 512], F32R, tag="xs", bufs=CFG["xs_bufs"], name="xs")
                for h in range(4):
                    nc.sync.dma_start(
                        out=st[:, h * 1024 : (h + 1) * 1024].rearrange(
                            "p (kt s) -> p kt s", s=512
                        ),
                        in_=xd.ap()[
                            h * 256 : (h + 1) * 256, qc * 512 : (qc + 1) * 512
                        ].rearrange("(kt p) s -> p kt s", p=128),
                    )
                return [st[:, kt * 512 : (kt + 1) * 512] for kt in range(KT)]

            # The K-projection inputs go out first so the PE isn't stuck
            # behind the (slow, latency-bound) staging DMAs at kernel start.
            # Alternate the weight/strip sub-DMAs so the first kt tiles of
            # both land as early as possible.
            wkt = rot.tile([128, KT * GH], F32R, tag="w", bufs=3, name="w")
            xkt = rot.tile([128, KT * 512], F32R, tag="xs", bufs=CFG["xs_bufs"], name="xs")
            for h in range(4):
                nc.sync.dma_start(
                    out=wkt[:, h * 2 * GH : (h + 1) * 2 * GH].rearrange(
                        "p (kt g) -> p kt g", g=GH
                    ),
                    in_=wkd.ap()[h * 256 : (h + 1) * 256, :].rearrange(
                        "(kt p) g -> p kt g", p=128
                    ),
                )
                nc.sync.dma_start(
                    out=xkt[:, h * 1024 : (h + 1) * 1024].rearrange(
                        "p (kt s) -> p kt s", s=512
                    ),
                    in_=xk.ap()[h * 256 : (h + 1) * 256, 0:512].rearrange(
                        "(kt p) s -> p kt s", p=128
                    ),
                )
            wk_sb = [wkt[:, kt * GH : (kt + 1) * GH] for kt in range(KT)]
            xk0 = [xkt[:, kt * 512 : (kt + 1) * 512] for kt in range(KT)]

            xk1 = load_strips(xk, 1)

            # ---- constants / weights staging ----
            nc.sync.dma_start(
                out=bq_sb, in_=bqd.ap().rearrange("(m p) -> p m", p=128)
            )
            for t in range(MT):
                nc.sync.dma_start(
                    out=wo_bf[t], in_=wod.ap()[t * 128 : (t + 1) * 128, :]
                )
            # ones columns of vaug only (V slots are written by the V proj)
            nc.vector.memset(
                vaug.rearrange("p (s c) -> p s c", c=VW)[:, :, D : D + 1], 1.0
            )

            wv_sb = load_w(wvd)

            # ---- V projection for one 128-key tile: vaug[kseq, head*65] ----
            def v_proj_sq(xs, qc, sql):
                sq = qc * 4 + sql
                ps = psmm.tile([128, 512], F32, tag="mm", name=f"psv{sq}")
                for kt in range(KT):
                    nc.tensor.matmul(
                        ps,
                        lhsT=xs[kt][:, sql * 128 : (sql + 1) * 128],
                        rhs=wv_sb[kt],
                        start=(kt == 0),
                        stop=(kt == KT - 1),
                    )
                base = sq * 8 * VW
                nc.vector.tensor_copy(
                    vaug[:, base : base + 8 * VW].rearrange(
                        "p (h c) -> p h c", c=VW
                    )[:, :, 0:D],
                    ps.rearrange("p (h c) -> p h c", c=D),
                )

            # ---- K projection: khT[m] = (xk @ wk)^T slice, f32r ----
            for qc in range(NQC):
                xs = (
                    xk0 if qc == 0
                    else xk1 if qc == 1
                    else load_strips(xk, qc)
                )
                for m in range(MT):
                    ps = psmm.tile([128, 512], F32, tag="mm", name=f"psk{m}")
                    for kt in range(KT):
                        nc.tensor.matmul(
                            ps,
                            lhsT=wk_sb[kt][:, m * 128 : (m + 1) * 128],
                            rhs=xs[kt],
                            start=(kt == 0),
                            stop=(kt == KT - 1),
                        )
                    nc.vector.tensor_copy(khT[m][:, qc * 512 : (qc + 1) * 512], ps)

            # ---- Q projection (+ bias) ----
            wq_sb = load_w(wqd)

            def q_proj_m(xs, qc, m):
                ps = psmm.tile([128, 512], F32, tag="mm", name=f"psq{m}")
                for kt in range(KT):
                    nc.tensor.matmul(
                        ps,
                        lhsT=wq_sb[kt][:, m * 128 : (m + 1) * 128],
                        rhs=xs[kt],
                        start=(kt == 0),
                        stop=(kt == KT - 1),
                    )
                nc.vector.tensor_scalar(
                    qhT[m][:, qc * 512 : (qc + 1) * 512],
                    ps,
                    bq_sb[:, m : m + 1],
                    None,
                    OP.add,
                )

            def q_proj(qc):
                xs = load_strips(xq, qc)
                for m in range(MT):
                    q_proj_m(xs, qc, m)

            # Filler queue: small PE work units dribbled one-per-kt into the
            # attention score loops, where the scalar engine is the bottleneck
            # and the PE has slack. Block-emitting these instead would starve
            # the scalar engine (PE is in-order) or stall the PE at phase
            # boundaries.
            filler = []

            def pump(kt):
                if kt % 2 == 1 and filler:
                    filler.pop(0)()

            # ---- attention for head pair t over a 1024-wide query chunk ----
            # extra(kt) lets the caller thread other PE work (V projection
            # chunks) into the score loop; it is emitted before scores(kt) so
            # the pending-AV pops (4 kt behind) always follow their V chunk.
            def attention(t, qcp, extra=None, act_norm=False):
                q0 = qcp * 1024
                ps_o = [
                    pso.tile([VW, 1024], F32, tag="o", name=f"pso{hh}")
                    for hh in range(2)
                ]
                pending = []  # (kt, [pt_h0, pt_h1]) awaiting AV
                def emit_av(kt, pts):
                    for hh in range(2):
                        h_abs = 2 * t + hh
                        vbase = kt * 8 * VW + h_abs * VW
                        for qch in range(2):
                            nc.tensor.matmul(
                                ps_o[hh][:, qch * 512 : (qch + 1) * 512],
                                lhsT=vaug[:, vbase : vbase + VW],
                                rhs=pts[hh][:, qch * 512 : (qch + 1) * 512],
                                start=(kt == 0),
                                stop=(kt == SQ - 1),
                            )
                for kt in range(SQ):
                    if extra is not None:
                        extra(kt)
                    pump(kt)
                    ps_ss = []
                    for hh in range(2):
                        hp = 64 * hh
                        ps_s = psmm.tile([128, 1024], F32, tag="mm", name="pss")
                        ps_ss.append(ps_s)
                        for qch in range(2):
                            nc.tensor.matmul(
                                ps_s[:, qch * 512 : (qch + 1) * 512],
                                lhsT=khT[t][hp : hp + 64, kt * 128 : (kt + 1) * 128],
                                rhs=qhT[t][
                                    hp : hp + 64, q0 + qch * 512 : q0 + (qch + 1) * 512
                                ],
                                start=True,
                                stop=True,
                            )
                    pts = []
                    for hh in range(2):
                        pt_t = rot.tile([128, 1024], BF16, tag="pt", bufs=CFG["pt_bufs"], name="pt")
                        nc.scalar.activation(pt_t, ps_ss[hh], AF.Exp, scale=SCALE)
                        pts.append(pt_t)
                    pending.append((kt, pts))
                    if len(pending) > 3:
                        emit_av(*pending.pop(0))
                for p in pending:
                    emit_av(*p)
                # Fast normalization for the final head pair: reciprocal as
                # exp(-ln(x)) on the (tail-idle) scalar engine straight from
                # PSUM — two fewer DMA-latency hops on the one chain whose
                # latency is exposed. Ln and Exp live in the same activation
                # table set, so this costs no table reloads.
                rws = {}
                if act_norm:
                    for hh in (1, 0):
                        rl = rot.tile([VW, 1024], F32, tag="rw", bufs=2, name="rl")
                        nc.scalar.activation(
                            rl[D : D + 1, :], ps_o[hh][D : D + 1, :], AF.Ln, scale=1.0
                        )
                        rw = rot.tile([VW, 1024], F32, tag="rw", bufs=2, name="rw")
                        nc.scalar.activation(
                            rw[D : D + 1, :], rl[D : D + 1, :], AF.Exp, scale=-1.0
                        )
                        rws[hh] = rw
                # Drain both accumulators to SBUF: the PSUM banks free as soon
                # as the copies retire, so the next head pair's AV starts
                # while the normalization below is still running. The den rows
                # go to DRAM right behind each drain (DMA cannot read PSUM).
                ods, scs = [], []
                for hh in range(2):
                    od = rot.tile([VW, 1024], F32, tag="od", bufs=CFG["od_bufs"], name="od")
                    nc.vector.tensor_copy(od, ps_o[hh])
                    ods.append(od)
                    if not act_norm:
                        sc = dsc.tile([1, 1024], F32, tag="sc", name="sc")
                        nc.sync.dma_start(out=sc, in_=od[D : D + 1, :])
                        scs.append(sc)
                # normalize by the ones-column sums; heads stack on partitions.
                # hh1 first: its path to the out-projection has an extra
                # partition-shift DMA hop, so it gets a head start.
                for hh in (1, 0):
                    od = ods[hh]
                    sc2 = dsc.tile([1, 1024], F32, tag="sc2", name="sc2")
                    if act_norm:
                        nc.sync.dma_start(out=sc2, in_=rws[hh][D : D + 1, :])
                    else:
                        # Reciprocal cost on DVE scales with free-size, so
                        # running it on the [1,1024] den row costs 6.5us and
                        # stalls every consumer queued behind it. Instead
                        # bounce the row through DRAM into a [64,16] spread,
                        # recip there (~0.3us), bounce back, then broadcast
                        # (DRAM-source DMA does the 0-stride partition reads;
                        # SBUF source does not support that).
                        dsp = rot.tile([64, 16], F32, tag="dsp", bufs=2, name="dsp")
                        nc.sync.dma_start(
                            out=dsp,
                            in_=scs[hh][0, :].rearrange("(p f) -> p f", f=16),
                        )
                        rcp = rot.tile([64, 16], F32, tag="rcp2", bufs=2, name="rcp")
                        nc.vector.reciprocal(rcp, dsp)
                        nc.sync.dma_start(
                            out=sc2[0, :].rearrange("(p f) -> p f", f=16), in_=rcp
                        )
                    bc = rot.tile([64, 1024], F32, tag="bc", bufs=2, name="bc")
                    nc.sync.dma_start(
                        out=bc, in_=sc2[0, :].partition_broadcast(64)
                    )
                    if hh == 0:
                        nc.vector.tensor_tensor(
                            oT[t][0:64, q0 : q0 + 1024],
                            od[0:D, :],
                            bc,
                            OP.mult,
                        )
                    else:
                        # normalized h1 lands on partitions 0-63; DMA shifts it
                        # onto partitions 64-127 of the head-pair tile
                        otn = rot.tile([64, 1024], BF16, tag="otn", bufs=2, name="otn")
                        nc.vector.tensor_tensor(
                            otn, od[0:D, :], bc, OP.mult
                        )
                        nc.sync.dma_start(
                            out=oT[t][64:128, q0 : q0 + 1024], in_=otn
                        )

            # ---- output projection, one 128-row m-tile of a 1024 chunk ----
            def out_proj_m(qp, m):
                ps = psmm.tile([128, 1024], F32, tag="mm", name=f"pso{m}")
                for qch in range(2):
                    for t in range(MT):
                        nc.tensor.matmul(
                            ps[:, qch * 512 : (qch + 1) * 512],
                            lhsT=wo_bf[t][:, m * 128 : (m + 1) * 128],
                            rhs=oT[t][
                                :, qp * 1024 + qch * 512 : qp * 1024 + (qch + 1) * 512
                            ],
                            start=(t == 0),
                            stop=(t == MT - 1),
                        )
                osb = rot.tile([128, 1024], BF16, tag="osb", bufs=3, name="osb")
                nc.vector.tensor_copy(osb, ps)
                nc.sync.dma_start(
                    out=otd.ap()[m * 128 : (m + 1) * 128, qp * 1024 : (qp + 1) * 1024],
                    in_=osb,
                )

            # V projection threaded into head pair 0's score loop at sq (128-
            # key) granularity: unit kt produces vaug tile kt; the earliest AV
            # touching it pops 4 iterations later, safely behind it.
            vxs = [None]

            def v_interleave(kt):
                if kt % 4 == 0:
                    vxs[0] = load_strips(xv, kt // 4)
                v_proj_sq(vxs[0], kt // 4, kt % 4)

            q_proj(0)
            q_proj(1)
            attention(0, 0, extra=v_interleave)
            for qc in (2, 3):
                xs_q = load_strips(xq, qc)
                for m in range(MT):
                    filler.append(
                        lambda xs_q=xs_q, qc=qc, m=m: q_proj_m(xs_q, qc, m)
                    )
            attention(1, 0)
            attention(2, 0)
            attention(3, 0)
            attention(0, 1)
            for m in range(H // 128 - 2):
                filler.append(lambda m=m: out_proj_m(0, m))
            attention(1, 1)
            attention(2, 1)
            attention(3, 1)
            while filler:
                filler.pop(0)()
            # The last two first-block units were held back: they depend only
            # on the long-finished qcp0 outputs, so they cover the front of
            # the final normalization chain.
            out_proj_m(0, H // 128 - 2)
            out_proj_m(0, H // 128 - 1)
            # Tail out-projection in (partial, finish) pairs: the t0-2
            # accumulation runs during the last head pair's normalization
            # chain; only the t3 matmuls wait on it.
            def op_partial(m, pool, tag):
                ps = pool.tile([128, 1024], F32, tag=tag, name=f"pst{m}")
                for qch in range(2):
                    for t in range(MT - 1):
                        nc.tensor.matmul(
                            ps[:, qch * 512 : (qch + 1) * 512],
                            lhsT=wo_bf[t][:, m * 128 : (m + 1) * 128],
                            rhs=oT[t][:, 1024 + qch * 512 : 1024 + (qch + 1) * 512],
                            start=(t == 0),
                            stop=False,
                        )
                return ps

            def op_finish(m, ps):
                for qch in range(2):
                    nc.tensor.matmul(
                        ps[:, qch * 512 : (qch + 1) * 512],
                        lhsT=wo_bf[MT - 1][:, m * 128 : (m + 1) * 128],
                        rhs=oT[MT - 1][:, 1024 + qch * 512 : 1024 + (qch + 1) * 512],
                        start=False,
                        stop=True,
                    )
                osb = rot.tile([128, 1024], BF16, tag="osb", bufs=3, name="osb")
                nc.vector.tensor_copy(osb, ps)
                nc.sync.dma_start(
                    out=otd.ap()[m * 128 : (m + 1) * 128, 1024:2048], in_=osb
                )

            # Four concurrent partial groups (both PSUM pools; the pso slots
            # are free once the final drains retire) keep the PE running
            # through the last normalization chain.
            for base in (0, 4):
                pss = [
                    op_partial(base + 0, psmm, "mm"),
                    op_partial(base + 1, psmm, "mm"),
                    op_partial(base + 2, pso, "o"),
                    op_partial(base + 3, pso, "o"),
                ]
                for j, ps2 in enumerate(pss):
                    op_finish(base + j, ps2)

    nc.compile()
    return nc


def _get_nc():
    if "nc" not in _CACHE:
        _CACHE["nc"] = _build()
    return _CACHE["nc"]


def make_in_maps(q, k, v, wq, wk, wv, wo, bq):
    q = np.asarray(q, np.float32)
    k = np.asarray(k, np.float32)
    v = np.asarray(v, np.float32)
    in_maps = []
    for c in range(NCORES):
        g, b = divmod(c, B)
        sl = slice(g * GH, (g + 1) * GH)
        in_maps.append(
            {
                "xq": np.ascontiguousarray(q[b].T).astype(np.float16),
                "xk": np.ascontiguousarray(k[b].T).astype(np.float16),
                "xv": np.ascontiguousarray(v[b].T).astype(np.float16),
                "wq": np.ascontiguousarray(np.asarray(wq, np.float32)[:, sl]).astype(np.float16),
                "wk": np.ascontiguousarray(np.asarray(wk, np.float32)[:, sl]).astype(np.float16),
                "wv": np.ascontiguousarray(np.asarray(wv, np.float32)[:, sl]).astype(np.float16),
                "wo": np.ascontiguousarray(np.asarray(wo, np.float32)[sl, :]).astype(np.float16),
                "bq": np.ascontiguousarray(np.asarray(bq, np.float32)[sl]),
            }
        )
    return in_maps


def assemble(per_core_ot, bv, wo, bo):
    bo_eff = (
        np.asarray(bo, np.float32)
        + np.asarray(bv, np.float32) @ np.asarray(wo, np.float32)
    )
    out = np.empty((B, S, H), np.float32)
    for b in range(B):
        out[b] = (
            per_core_ot[b].T.astype(np.float32)
            + per_core_ot[B + b].T.astype(np.float32)
            + bo_eff
        )
    return out


def kernel(q, k, v, wq, bq, wk, bk, wv, bv, wo, bo, _trace=False):
    from concourse.bass_utils import run_bass_kernel_spmd

    nc = _get_nc()
    in_maps = make_in_maps(q, k, v, wq, wk, wv, wo, bq)
    res = run_bass_kernel_spmd(
        nc, in_maps, core_ids=list(range(NCORES)), trace=_trace
    )
    _CACHE["last_results"] = res
    outs = [res.results[c]["ot"] for c in range(NCORES)]
    return assemble(outs, bv, wo, bo)
